# revision 35
# baseline (speedup 1.0000x reference)
"""Trainium2 Bass kernel for nn_DiffusionNCA_fft2 (B=32, S=64, C=32, HID=256).

Self-contained: takes FULL inputs (as from setup_inputs()), shards batch over
8 NeuronCores (4 per core), runs one SPMD Bass program, gathers FULL output.

Device pipeline per batch element (all layouts validated in numpy first):
  fft2 via DFT matmuls (stacked re/im on partitions) with DRAM-bounce
  reshuffles -> dx (reflect-padded) -> 3x3 convs as 6 offset-paired matmuls
  -> fc0 (+folded conv/fc0 biases) + LeakyReLU fused into ACT psum-evac with
  running sums -> LN stats (sq-pass on GPSIMD) -> ln_w fold (DVE) -> fc1
  (2-batch col-tiled) with LN affine folded across it -> stochastic mask ->
  inverse DFT matmuls -> output = x + ifft2(masked update) (x added on host,
  exact).
"""

import os
from contextlib import ExitStack

import numpy as np
import ml_dtypes

import concourse.bass as bass
import concourse.mybir as mybir
import concourse.tile as tile
from concourse import bacc
from concourse import bass_isa

S = 64
C = 32
C2 = 64
C6 = 192
HID = 256
B = 32
NCORES = 8
BPC = B // NCORES            # batch per core
SP = 66                      # padded spatial
NPAD = SP * SP               # 4356
NPIX = S * S                 # 4096
LN_N = float(HID * NPIX)     # LN element count per batch
EPS = 1e-5
FIRE = 0.5

f32 = mybir.dt.float32
f32r = mybir.dt.float32r
bf16 = mybir.dt.bfloat16
AF = mybir.ActivationFunctionType
ALU = mybir.AluOpType

_BF = ml_dtypes.bfloat16


def _dft_mats():
    t = np.arange(S)
    ang = -2.0 * np.pi * np.outer(t, t) / S
    return np.cos(ang).astype(np.float32), np.sin(ang).astype(np.float32)


def host_constants(inp):
    """All per-core constant inputs, in device layouts (shared by all cores)."""
    Fr, Fi = _dft_mats()
    cst = {}

    ff1 = np.zeros((S, 2 * S), np.float32)
    ff1[:, :S], ff1[:, S:] = Fr.T, Fi.T
    cst["ff1"] = ff1.astype(_BF)

    w2 = np.zeros((2 * S, 2 * S), np.float32)
    w2[:S, :S], w2[S:, :S] = Fr.T, -Fi.T
    w2[:S, S:], w2[S:, S:] = Fi.T, Fr.T
    cst["w2"] = w2.astype(_BF)

    Gr, Gi = Fr / S, -Fi / S
    wa = np.zeros((2 * S, 2 * S), np.float32)
    wa[:S, :S], wa[S:, :S] = Gr.T, -Gi.T
    wa[:S, S:], wa[S:, S:] = Gi.T, Gr.T
    cst["wa"] = wa.astype(_BF)

    a = np.linspace(1.0, 0.0, S, dtype=np.float32)
    alive = (a[:, None] + a[None, :]) * 0.5
    cst["alive"] = np.pad(alive, 1, mode="reflect").reshape(-1).astype(_BF)

    p0w, p1w = np.asarray(inp["p0_w"]), np.asarray(inp["p1_w"])
    wpair = np.zeros((2 * C2, 3 * 2 * C2), np.float32)  # [k, di*128 + m]
    wsing = np.zeros((C2, 3 * 2 * C2), np.float32)
    for di in range(3):
        mo = di * 2 * C2
        wpair[:C2, mo:mo + C2] = p0w[:, :, di, 0].T
        wpair[C2:, mo:mo + C2] = p0w[:, :, di, 1].T
        wpair[:C2, mo + C2:mo + 2 * C2] = p1w[:, :, di, 0].T
        wpair[C2:, mo + C2:mo + 2 * C2] = p1w[:, :, di, 1].T
        wsing[:, mo:mo + C2] = p0w[:, :, di, 2].T
        wsing[:, mo + C2:mo + 2 * C2] = p1w[:, :, di, 2].T
    cst["wpair"] = wpair.astype(_BF)
    cst["wsing"] = wsing.astype(_BF)

    fc0w = np.asarray(inp["fc0_w"])
    cst["fc0a"] = fc0w[:C2].astype(_BF)          # [64, 256]
    cst["fc0bb"] = fc0w[C2:].astype(_BF)         # [128, 256]
    fc0b = (np.asarray(inp["fc0_b"])
            + np.asarray(inp["p0_b"]) @ fc0w[C2:2 * C2]
            + np.asarray(inp["p1_b"]) @ fc0w[2 * C2:])
    cst["fc0b2"] = fc0b.reshape(2, 128).T.astype(np.float32).copy()  # [128, 2]

    fc1w = np.asarray(inp["fc1_w"]).astype(np.float32)  # [256, 64]
    fc1t = np.zeros((128, 128), np.float32)
    fc1t[:, :64], fc1t[:, 64:] = fc1w[:128], fc1w[128:]
    cst["fc1"] = fc1t.astype(_BF)

    lnw = np.asarray(inp["ln_w"]).astype(np.float32)
    lnb = np.asarray(inp["ln_b"]).astype(np.float32)
    lnw_dev = np.transpose(lnw, (2, 1, 0)).reshape(HID, NPIX)  # [k, (a,b)]
    lnb_dev = np.transpose(lnb, (2, 1, 0)).reshape(HID, NPIX)
    cst["lnw"] = np.concatenate([lnw_dev[:128], lnw_dev[128:]], axis=1).astype(_BF)  # [128, 8192]
    lw1 = fc1w[:128].T @ lnw_dev[:128] + fc1w[128:].T @ lnw_dev[128:]  # [64, 4096]
    lb1 = fc1w[:128].T @ lnb_dev[:128] + fc1w[128:].T @ lnb_dev[128:]
    cst["lw1t"] = np.concatenate([lw1, lw1], axis=0).astype(_BF)  # [128, 4096] (2b dup)
    cst["lbt"] = np.concatenate([lb1, lb1], axis=0).astype(_BF)
    return cst


def build_nc(steps=1):
    nc = bacc.Bacc("TRN2", target_bir_lowering=False, debug=False)

    # ---- I/O ----
    xs = nc.dram_tensor("xs", [BPC, S, S, C], bf16, kind="ExternalInput")
    ins = {}
    cshape = dict(ff1=([S, 2 * S], bf16), w2=([2 * S, 2 * S], bf16),
                  wa=([2 * S, 2 * S], bf16), alive=([NPAD], bf16),
                  wpair=([2 * C2, 3 * 2 * C2], bf16), wsing=([C2, 3 * 2 * C2], bf16),
                  fc0a=([C2, HID], bf16), fc0bb=([2 * C2, HID], bf16),
                  fc0b2=([128, 2], f32), fc1=([128, 128], bf16),
                  lnw=([128, 2 * NPIX], bf16), lw1t=([128, NPIX], bf16),
                  lbt=([128, NPIX], bf16))
    for name, (shp, dt) in cshape.items():
        ins[name] = nc.dram_tensor(name, shp, dt, kind="ExternalInput")
    maskd = nc.dram_tensor("maskd", [BPC // 2, 128, NPIX], bf16, kind="ExternalInput")

    D1 = nc.dram_tensor("D1", [BPC, 2 * S, S * C], bf16)
    D2 = nc.dram_tensor("D2", [BPC, 2 * S, C * S], bf16)
    D3 = nc.dram_tensor("D3", [BPC // 2, 2, 2 * S, C * S], bf16)
    D4 = nc.dram_tensor("D4", [BPC, 2 * S, S * C], bf16)
    OUT = nc.dram_tensor("OUT", [BPC, 2 * S, S * C], bf16, kind="ExternalOutput")

    with tile.TileContext(nc) as tc, ExitStack() as ctx:
        cpool = ctx.enter_context(tc.tile_pool(name="consts", bufs=1))
        xpool = ctx.enter_context(tc.tile_pool(name="x", bufs=2))
        fpool4 = ctx.enter_context(tc.tile_pool(name="fftw4", bufs=4))
        fpool = ctx.enter_context(tc.tile_pool(name="fftw", bufs=1))
        mpool = ctx.enter_context(tc.tile_pool(name="maskp", bufs=2))
        f1pool = ctx.enter_context(tc.tile_pool(name="fftw1", bufs=1))
        dxpool = ctx.enter_context(tc.tile_pool(name="dx", bufs=4))
        ypool = ctx.enter_context(tc.tile_pool(name="yconv", bufs=1))
        hpool = ctx.enter_context(tc.tile_pool(name="h", bufs=4))
        spool = ctx.enter_context(tc.tile_pool(name="small", bufs=8))
        tpool = ctx.enter_context(tc.tile_pool(name="tail", bufs=1))
        t1pool = ctx.enter_context(tc.tile_pool(name="tail1", bufs=1))
        pfft = ctx.enter_context(tc.tile_pool(name="pfft", bufs=2, space="PSUM"))
        pmm = ctx.enter_context(tc.tile_pool(name="pmm", bufs=2, space="PSUM"))

        # ---- constants to SBUF ----
        ct = {}
        for name, (shp, dt) in cshape.items():
            if name == "alive":
                continue
            t = cpool.tile(shp, dt, tag="c_" + name)
            nc.sync.dma_start(t[:], ins[name][:])
            ct[name] = t

        ones = cpool.tile([128, 128], f32, tag="c_ones")
        nc.gpsimd.memset(ones[:], 1.0)

        # per-b state kept across pair phases
        h_tiles = {}     # (b, m) -> [128, 4096] bf16 (becomes hw in-place)
        stats = {}       # b -> dict of [128,1] tiles
        dgath = {}       # b -> [128, 2048] bf16 update in freq-stacked layout
        upacc = {}       # b -> [128, 2048] f32 accumulated update (steps > 1)

        def fft_fwd(b, first):
            """x -> xf (S2 [(ri,u), (c,v)] bf16) + dump to D2[b]."""
            X = xpool.tile([S, S * C], bf16, tag="X")
            nc.sync.dma_start(X[:], xs[b].rearrange("a b c -> a (b c)"))
            if not first:
                # steps>1: xf comes from S2 + accumulated updates (handled
                # by caller rewriting S2); this branch unused for steps==1.
                pass
            # F1: psum1 [(ri,v), (s1,c)] in two 1024-halves
            t1d = fpool.tile([2 * S, S * C], bf16, tag="stageA", name=f"t1d_{b}")
            for half in range(2):
                ps = pfft.tile([2 * S, 1024], f32, tag="pfft")
                for q in range(2):
                    sl = bass.ts(half * 2 + q, 512)
                    nc.tensor.matmul(ps[:, bass.ts(q, 512)],
                                     ct["ff1"][:], X[:, sl])
                nc.vector.tensor_copy(t1d[:, bass.ts(half, 1024)], ps[:])
            nc.sync.dma_start(D1[b][:], t1d[:])
            # bounce 1 -> T1g [(ri,s1), (v,c)]; split per ri (3-dim AP limit)
            t1g = fpool4.tile([2 * S, S * C], bf16, tag="stageB", name=f"t1g_{b}")
            d1v = D1[b].rearrange("(ri v) (s1 c) -> ri s1 v c", ri=2, v=S, s1=S, c=C)
            for ri in range(2):
                nc.sync.dma_start(
                    t1g[bass.ts(ri, S), :].rearrange("p (v c) -> p v c", v=S, c=C),
                    d1v[ri])
            # F2 + evac reorder (v,c)->(c,v)
            s2 = f1pool.tile([2 * S, C * S], bf16, tag="s2")
            for half in range(2):
                ps = pfft.tile([2 * S, 1024], f32, tag="pfft")
                for q in range(2):
                    nc.tensor.matmul(ps[:, bass.ts(q, 512)], ct["w2"][:],
                                     t1g[:, bass.ds(half * 1024 + q * 512, 512)])
                # psum free = (v-half, c): v in [half*32, half*32+32)
                nc.scalar.copy(
                    s2[:].rearrange("p (c v) -> p v c", c=C, v=S)[:, bass.ts(half, 32), :],
                    ps[:].rearrange("p (v c) -> p v c", v=32, c=C))
            nc.sync.dma_start(D2[b][:], s2[:])

        def build_dx(b):
            dx2 = dxpool.tile([2 * C2, NPAD], bf16, tag="dx2")
            dxv = dx2[:, 0:NPAD].rearrange("q (a b) -> q a b", a=SP, b=SP)
            d2v = D2[b].rearrange("(ri u) (c v) -> ri c u v", ri=2, u=S, c=C, v=S)
            # interiors split across the two HWDGE rings (SP + ACT) to avoid
            # head-of-line blocking; reflect pads via cheap DVE copies.
            nc.sync.dma_start(dxv[0:32, 1:S + 1, 1:S + 1], d2v[0])
            nc.scalar.dma_start(dxv[32:64, 1:S + 1, 1:S + 1], d2v[1])
            nc.sync.dma_start(dx2[C2 - 1:C2, 0:NPAD], ins["alive"][None, :])
            q = slice(0, C2 - 1)
            nc.vector.tensor_copy(dxv[q, 1:S + 1, 0:1], dxv[q, 1:S + 1, 2:3])
            nc.vector.tensor_copy(dxv[q, 1:S + 1, SP - 1:SP],
                                  dxv[q, 1:S + 1, SP - 3:SP - 2])
            nc.vector.tensor_copy(dxv[q, 0:1, :], dxv[q, 2:3, :])
            nc.vector.tensor_copy(dxv[q, SP - 1:SP, :], dxv[q, SP - 3:SP - 2, :])
            # B-half (partitions 64:127 = dx_pad shifted +1 in flat free; only
            # cols 0:64 of each padded row are ever read by the paired convs).
            nc.sync.dma_start(dxv[64:96, 1:S + 1, 0:S], d2v[0])
            nc.scalar.dma_start(dxv[96:128, 1:S + 1, 0:S], d2v[1])
            nc.scalar.dma_start(dx2[2 * C2 - 1:2 * C2, 0:NPAD - 1],
                                ins["alive"][None, 1:NPAD])
            qb = slice(C2, 2 * C2 - 1)
            nc.vector.tensor_copy(dxv[qb, 0:1, 0:S], dxv[qb, 2:3, 0:S])
            nc.vector.tensor_copy(dxv[qb, SP - 1:SP, 0:S],
                                  dxv[qb, SP - 3:SP - 2, 0:S])
            return dx2

        def conv_fc0(b, dx2):
            dxv = dx2[:, 0:NPAD].rearrange("q (a b) -> q a b", a=SP, b=SP)
            s1cols = spool.tile([128, 8], f32, tag="s1cols")
            s2cols = spool.tile([128, 16], f32, tag="s2cols")
            for m in range(2):
                h_tiles[(b, m)] = hpool.tile([128, NPIX], bf16, tag="h", name=f"h_{b}_{m}")
            for T in range(4):
                r0 = T * 16
                psy = pmm.tile([2 * C2, 1024], f32, tag="pmm")
                for q in range(2):
                    rq = r0 + q * 8
                    for di in range(3):
                        nc.tensor.matmul(
                            psy[:, bass.ts(q, 512)],
                            ct["wpair"][:, bass.ts(di, 2 * C2)],
                            dxv[:, rq + di:rq + di + 8, 0:S],
                            start=(di == 0), stop=False)
                    for di in range(3):
                        nc.tensor.matmul(
                            psy[:, bass.ts(q, 512)],
                            ct["wsing"][:, bass.ts(di, 2 * C2)],
                            dxv[0:C2, rq + di:rq + di + 8, 2:SP],
                            start=False, stop=(di == 2))
                yc = ypool.tile([2 * C2, 1024], bf16, tag="yc")
                eng = nc.vector.tensor_copy if T % 2 == 0 else nc.scalar.copy
                eng(yc[:], psy[:])
                for m in range(2):
                    psh = pmm.tile([128, 1024], f32, tag="pmm")
                    for q in range(2):
                        rq = r0 + q * 8
                        nc.tensor.matmul(psh[:, bass.ts(q, 512)],
                                         ct["fc0a"][:, bass.ts(m, 128)],
                                         dxv[0:C2, rq + 1:rq + 9, 1:S + 1],
                                         start=True, stop=False)
                        nc.tensor.matmul(psh[:, bass.ts(q, 512)],
                                         ct["fc0bb"][:, bass.ts(m, 128)],
                                         yc[:, bass.ts(q, 512)],
                                         start=False, stop=True)
                    nc.scalar.activation(
                        h_tiles[(b, m)][:, bass.ts(T, 1024)], psh[:],
                        AF.Lrelu, bias=ct["fc0b2"][:, m:m + 1], scale=1.0,
                        alpha=0.01, accum_out=s1cols[:, T * 2 + m:T * 2 + m + 1])
            # sq-pass (DVE) + running sums
            for m in range(2):
                for t in range(8):
                    scr = t1pool.tile([128, 512], bf16, tag="sqscr",
                                      name=f"scr_{b}_{m}_{t}")
                    hs = h_tiles[(b, m)][:, bass.ts(t, 512)]
                    nc.vector.scalar_tensor_tensor(
                        out=scr[:], in0=hs, scalar=0.0, in1=hs,
                        op0=ALU.bypass, op1=ALU.mult,
                        accum_out=s2cols[:, m * 8 + t:m * 8 + t + 1])
            stats2 = spool.tile([128, 2], f32, tag="stats2", name=f"stats2_{b}")
            nc.vector.tensor_reduce(stats2[:, 0:1], s1cols[:], axis=mybir.AxisListType.X,
                                    op=ALU.add)
            nc.vector.tensor_reduce(stats2[:, 1:2], s2cols[:], axis=mybir.AxisListType.X,
                                    op=ALU.add)
            pst = pmm.tile([128, 2], f32, tag="pmm", name=f"pst_{b}")
            nc.tensor.matmul(pst[:], ones[:], stats2[:])
            mu = spool.tile([128, 1], f32, tag="stat")
            nc.scalar.mul(mu[:], pst[:, 0:1], 1.0 / LN_N)
            msq = spool.tile([128, 1], f32, tag="stat")
            nc.vector.tensor_mul(msq[:], mu[:], mu[:])
            var = spool.tile([128, 1], f32, tag="stat")
            nc.vector.scalar_tensor_tensor(out=var[:], in0=pst[:, 1:2],
                                           scalar=1.0 / LN_N, in1=msq[:],
                                           op0=ALU.mult, op1=ALU.subtract)
            nc.vector.tensor_scalar_add(var[:], var[:], EPS)
            sd = spool.tile([128, 1], f32, tag="stat")
            nc.scalar.activation(sd[:], var[:], AF.Sqrt, bias=0.0, scale=1.0)
            r = spool.tile([128, 1], f32, tag="stat")
            nc.vector.reciprocal(r[:], sd[:])
            nrm = spool.tile([128, 1], f32, tag="stat")
            nc.vector.tensor_mul(nrm[:], r[:], mu[:])
            nc.scalar.mul(nrm[:], nrm[:], -1.0)
            stats[b] = {"r": r, "nrm": nrm}
            # hw-pass in place: h <- h * ln_w
            for m in range(2):
                nc.vector.tensor_mul(h_tiles[(b, m)][:], h_tiles[(b, m)][:],
                                     ct["lnw"][:, bass.ts(m, NPIX)])

        def fc1_tail(pair):
            b0, b1 = 2 * pair, 2 * pair + 1
            r2 = spool.tile([128, 1], f32, tag="stat")
            nrm2 = spool.tile([128, 1], f32, tag="stat")
            nc.vector.tensor_copy(r2[0:64, :], stats[b0]["r"][0:64, :])
            nc.vector.tensor_copy(r2[64:128, :], stats[b1]["r"][64:128, :])
            nc.vector.tensor_copy(nrm2[0:64, :], stats[b0]["nrm"][0:64, :])
            nc.vector.tensor_copy(nrm2[64:128, :], stats[b1]["nrm"][64:128, :])
            z = t1pool.tile([128, NPIX], bf16, tag="ztile")
            nc.vector.scalar_tensor_tensor(
                out=z[:], in0=ct["lw1t"][:], scalar=nrm2[:], in1=ct["lbt"][:],
                op0=ALU.mult, op1=ALU.add)
            mask2 = mpool.tile([128, NPIX], bf16, tag="mask2", name=f"mask2_{pair}")
            nc.sync.dma_start(mask2[:], maskd[pair][:])
            dm = tpool.tile([128, NPIX], bf16, tag="dm")
            for T in range(4):
                psda = pmm.tile([128, 1024], f32, tag="pmm", name=f"psda_{pair}_{T}")
                psdb = pmm.tile([128, 1024], f32, tag="pmm", name=f"psdb_{pair}_{T}")
                for q in range(2):
                    for m in range(2):
                        for half, b, pt in ((0, b0, psda), (1, b1, psdb)):
                            nc.tensor.matmul(
                                pt[bass.ts(half, 64), bass.ts(q, 512)],
                                ct["fc1"][:, bass.ts(m, 64)],
                                h_tiles[(b, m)][:, bass.ds(T * 1024 + q * 512, 512)],
                                start=(m == 0), stop=(m == 1),
                                tile_position=(0, half * 64))
                for half, pt in ((0, psda), (1, psdb)):
                    hs = bass.ts(half, 64)
                    nc.vector.scalar_tensor_tensor(
                        out=dm[hs, bass.ts(T, 1024)], in0=pt[hs, :],
                        scalar=r2[hs, :], in1=z[hs, bass.ts(T, 1024)],
                        op0=ALU.mult, op1=ALU.add)
            nc.vector.tensor_mul(dm[:], dm[:], mask2[:])
            for hb in range(2):
                for ri in range(2):
                    # dump in [ri, u, c, v] layout per batch-half
                    nc.sync.dma_start(
                        D3[pair][hb].rearrange("(ri u) (c v) -> ri c u v",
                                               ri=2, u=S, c=C, v=S)[ri],
                        dm[bass.ds(hb * 64 + ri * 32, 32), :].rearrange(
                            "c (u v) -> c u v", u=S, v=S))
            for half, b in ((0, b0), (1, b1)):
                dg = fpool4.tile([2 * S, C * S], bf16, tag="dg", name=f"dg_{b}")
                d3g = D3[pair][half].rearrange("(ri u) (c v) -> ri u c v",
                                               ri=2, u=S, c=C, v=S)
                for ri in range(2):
                    nc.sync.dma_start(
                        dg[bass.ts(ri, S), :].rearrange("p (c v) -> p c v", c=C, v=S),
                        d3g[ri])
                dgath[b] = dg

        def ifft_out(b):
            upd = dgath[b]
            # IFFT-A: contract u
            sa = fpool.tile([2 * S, S * C], bf16, tag="stageA", name=f"sa_{b}")
            for half in range(2):
                ps = pfft.tile([2 * S, 1024], f32, tag="pfft")
                for q in range(2):
                    nc.tensor.matmul(ps[:, bass.ts(q, 512)], ct["wa"][:],
                                     upd[:, bass.ds(half * 1024 + q * 512, 512)])
                # psum free = (c-half, v) ; SA free = (v, c)
                nc.vector.tensor_copy(
                    sa[:].rearrange("p (v c) -> p c v", v=S, c=C)[:, bass.ts(half, 16), :],
                    ps[:].rearrange("p (c v) -> p c v", c=16, v=S))
            nc.sync.dma_start(D4[b][:], sa[:])
            dgb = fpool4.tile([2 * S, S * C], bf16, tag="stageB2", name=f"dgb_{b}")
            d4v = D4[b].rearrange("(ri a) (v c) -> ri v a c", ri=2, a=S, v=S, c=C)
            for ri in range(2):
                nc.sync.dma_start(
                    dgb[bass.ts(ri, S), :].rearrange("p (a c) -> p a c", a=S, c=C),
                    d4v[ri])
            sb = f1pool.tile([2 * S, S * C], bf16, tag="sb")
            for half in range(2):
                ps = pfft.tile([2 * S, 1024], f32, tag="pfft")
                for q in range(2):
                    nc.tensor.matmul(ps[:, bass.ts(q, 512)], ct["wa"][:],
                                     dgb[:, bass.ds(half * 1024 + q * 512, 512)])
                nc.scalar.copy(sb[:, bass.ts(half, 1024)], ps[:])
            nc.sync.dma_start(OUT[b][:], sb[:])

        assert steps == 1, "device program built for steps==1"
        # software-pipelined emission: issue b+1's FFT front-end before b's
        # conv/fc phase so the scheduler can fill DMA-wait gaps with matmuls.
        fft_fwd(0, first=True)
        for b in range(BPC):
            dx2 = build_dx(b)
            if b + 1 < BPC:
                fft_fwd(b + 1, first=True)
            conv_fc0(b, dx2)
            if b % 2 == 1:
                fc1_tail(b // 2)
                ifft_out(b - 1)
                ifft_out(b)

    return nc


_BUILT = {}


def kernel(**inputs):
    x = np.ascontiguousarray(np.asarray(inputs["x"], dtype=np.float32))
    steps = int(np.asarray(inputs["steps"]))
    if steps == 0:
        return x.astype(np.complex64)
    assert steps == 1, f"unsupported steps={steps}"

    cst = host_constants(inputs)
    su = np.asarray(inputs["stoch_u"], dtype=np.float32)[..., 0]   # [B, S, S]
    mask = (su > FIRE).astype(np.float32)
    mask_dev = np.ascontiguousarray(np.transpose(mask, (0, 2, 1))
                                    ).reshape(B, NPIX).astype(_BF)
    mask_pairs = np.empty((B // 2, 128, NPIX), _BF)
    for p in range(B // 2):
        mask_pairs[p, :64] = mask_dev[2 * p][None, :]
        mask_pairs[p, 64:] = mask_dev[2 * p + 1][None, :]

    if "nc" not in _BUILT:
        nc = build_nc(steps=1)
        nc.finalize()
        _BUILT["nc"] = nc
    nc = _BUILT["nc"]

    in_maps = []
    for core in range(NCORES):
        m = {k: np.ascontiguousarray(v) for k, v in cst.items()}
        m["xs"] = x[core * BPC:(core + 1) * BPC].astype(_BF)
        m["maskd"] = mask_pairs[core * (BPC // 2):(core + 1) * (BPC // 2)]
        in_maps.append(m)

    from concourse.bass_utils import run_bass_kernel_spmd
    trace = bool(int(os.environ.get("KERNEL_TRACE", "0")))
    res = run_bass_kernel_spmd(nc, in_maps, list(range(NCORES)), trace=trace)
    if trace and res.exec_time_ns is not None:
        print(f"HW exec time: {res.exec_time_ns} ns")
        if res.instructions_and_trace is not None:
            print("trace:", res.instructions_and_trace[1])

    out = np.empty((B, S, S, C), np.complex64)
    for core in range(NCORES):
        o = np.asarray(res.results[core]["OUT"], dtype=np.float32)  # [BPC,128,2048]
        for j in range(BPC):
            b = core * BPC + j
            re = o[j, :S].reshape(S, S, C)
            im = o[j, S:].reshape(S, S, C)
            out[b] = x[b] + re + 1j * im
    return out
